# revision 20
# baseline (speedup 1.0000x reference)
"""CQCNN piece estimator on 8 trn2 NeuronCores.

Strategy: pure data parallel over batch (8192 samples/core), SPMD (one NEFF).
Activations feature-major [features(partitions), batch(free)].
Convs are dense linear maps executed as fp8e4 DoubleRow matmuls (2 k-subtiles
per pass at 0.5 cycles/col) with zero-masked subtiles for odd k-tile counts.
conv2's bias rides a constant-1 row in the padded h1 tile so maxpool+bias+relu
collapses to 2 tensor_max + 1 scalar_tensor_tensor straight off PSUM.
Post-matmul work is spread across ACT / DVE / GPSIMD(Pool).
"""

import numpy as np
import ml_dtypes

import concourse.bass as bass
import concourse.bacc as bacc
import concourse.mybir as mybir
import concourse.tile as tile
from concourse.bass_utils import run_bass_kernel_spmd

BF16 = mybir.dt.bfloat16
F32 = mybir.dt.float32
FP8 = mybir.dt.float8e4
nbf = ml_dtypes.bfloat16
nf8 = ml_dtypes.float8_e4m3

B = 65536
NCORES = 8
BC = B // NCORES          # 8192 per core
CB = 2048                 # chunk of batch processed per pipeline pass
NCHUNK = BC // CB         # 4
NSL = 512                 # matmul moving-operand slice
NQ = 8

AF = mybir.ActivationFunctionType
ALU = mybir.AluOpType

_cache = {}

# ---- m-tile geometry ----
M1P = [(0, 128), (128, 256), (256, 384), (384, 512), (512, 576)]   # conv1 out
M2P = [(p * 288 + t * 96, p * 288 + (t + 1) * 96)
       for p in range(4) for t in range(3)]                        # conv2 out
M3P = [(0, 128), (128, 256), (256, 384), (384, 512), (512, 576)]   # conv3 out
MMP = [(0, 128), (128, 192)]                                       # mlp1 out

# engine assignment ('a'=ACT, 'd'=DVE, 'p'=Pool).
# Pool/GPSIMD cannot touch PSUM, and compute ops read at most ONE PSUM
# operand, so: ACT relu-copies even parities, DVE merges odd parities via
# scalar_tensor_tensor (1 PSUM input), Pool does the SBUF-only final max.
ENG_C1 = ['a', 'a', 'd', 'a', 'a']     # conv1 biasrelu per m-tile
ENG_C3 = ['a', 'd', 'a', 'd', 'd']     # conv3 biasrelu per m-tile
ENG_M1 = ['a', 'a']                    # mlp1 per m-tile
ENG_W2 = 'd'                           # w2 biasrelu


def _build_conv_maps(conv1_w, conv2_w, conv3_w):
    """Dense linear maps for the three convs (same orderings as baseline).

    X in-features  : channel-major c*36 + y*6 + x
    H1 out-features: pos-major (y*6+x)*16 + c
    H2 out-features: par*288 + q*32 + c   (par=(y%2)*2+x%2, q=(y//2)*3+x//2)
    P  (pooled)    : q*32 + c
    H3 out-features: pos-major (y*3+x)*64 + c
    """
    T1 = np.zeros((108, 576), np.float32)
    for co in range(16):
        for ci in range(3):
            for ky in range(3):
                for kx in range(3):
                    w = conv1_w[co, ci, ky, kx]
                    for yo in range(6):
                        yi = yo + ky - 1
                        if not 0 <= yi < 6:
                            continue
                        for xo in range(6):
                            xi = xo + kx - 1
                            if 0 <= xi < 6:
                                T1[ci * 36 + yi * 6 + xi, (yo * 6 + xo) * 16 + co] = w

    T2 = np.zeros((576, 1152), np.float32)
    for ky in range(3):
        for kx in range(3):
            w = conv2_w[:, :, ky, kx]  # [32,16]
            for yo in range(6):
                yi = yo + ky - 1
                if not 0 <= yi < 6:
                    continue
                for xo in range(6):
                    xi = xo + kx - 1
                    if 0 <= xi < 6:
                        par = (yo % 2) * 2 + (xo % 2)
                        qo = (yo // 2) * 3 + (xo // 2)
                        pi, po = (yi * 6 + xi) * 16, par * 288 + qo * 32
                        T2[pi:pi + 16, po:po + 32] = w.T

    T3 = np.zeros((288, 576), np.float32)
    for ky in range(3):
        for kx in range(3):
            w = conv3_w[:, :, ky, kx]  # [64,32]
            for yo in range(3):
                yi = yo + ky - 1
                if not 0 <= yi < 3:
                    continue
                for xo in range(3):
                    xi = xo + kx - 1
                    if 0 <= xi < 3:
                        pi, po = (yi * 3 + xi) * 32, (yo * 3 + xo) * 64
                        T3[pi:pi + 32, po:po + 64] = w.T
    return T1, T2, T3


def _pairs_from_run(ka, kb, kmax):
    """Pair a contiguous k-tile run [ka..kb] into DoubleRow (k0,k0+1) pairs.

    Returns [(k0, mask0, mask1)]; a masked subtile contributes zero weights
    (partner tile holds real data, so no NaN×0 risk)."""
    pairs = []
    i = ka
    while i <= kb:
        if i + 1 <= kb:
            pairs.append((i, 1, 1))
            i += 2
        else:
            if i + 1 <= kmax:
                pairs.append((i, 1, 0))
            else:
                pairs.append((i - 1, 0, 1))
            i += 1
    return pairs


def _conv2_pairs():
    """Per conv2 m-tile (p,t): DR pairs + which pair carries the bias row.

    Bias rides row 64 of k-tile 4 (h1big pad row holding constant 1.0)."""
    out = []
    for p in range(4):
        py = p // 2
        for t in range(3):
            yo = 2 * t + py
            a_pos, b_pos = 6 * max(0, yo - 1), 6 * min(5, yo + 1) + 5
            ka, kb = (a_pos * 16) // 128, (b_pos * 16 + 15) // 128
            pairs = [list(pr) + [False] for pr in _pairs_from_run(ka, kb, 4)]
            carrier = next((pr for pr in pairs if pr[0] == 3), None)
            if carrier is None:
                pairs.append([3, 0, 0, True])
            else:
                carrier[3] = True
            out.append(pairs)
    return out


def _conv3_pairs():
    # supports over 3 pool bands; bias rides row 96 of band 1 (pad const-1)
    out = []
    for mj in range(5):
        ys = {pos // 3 for pos in range(mj * 2, min(mj * 2 + 2, 9))}
        bands = sorted({b for y in ys for b in (y - 1, y, y + 1) if 0 <= b <= 2})
        pairs = [list(pr) + [False] for pr in _pairs_from_run(bands[0], bands[-1], 2)]
        carrier = next(pr for pr in pairs
                       if pr[0] == 1 or (pr[0] + 1 == 1))
        carrier[3] = True
        out.append(pairs)
    return out


PAIRS2 = _conv2_pairs()
PAIRS3 = _conv3_pairs()
PAIRS_M = [[0, 1, 1, False], [2, 1, 1, False], [3, 0, 1, True]]  # mlp1 k-pairs
# mlp1 bias rides row 72 of k-tile 4 (h3big pad const-1 row)


def _mk_layout():
    """fp8 stationary block offsets inside wb8 [128, 2, NC8H]."""
    off, c = {}, 0

    def a(name, cols):
        nonlocal c
        off[name] = c
        c += cols
    a("t1", 576)
    for mj in range(12):
        for i in range(len(PAIRS2[mj])):
            a(f"t2_{mj}_{i}", 96)
    for mj in range(5):
        for i in range(len(PAIRS3[mj])):
            a(f"t3_{mj}_{i}", M3P[mj][1] - M3P[mj][0])
    for mt in range(2):
        for i in range(3):
            a(f"w1_{mt}_{i}", MMP[mt][1] - MMP[mt][0])
    return off, c


OFF8, NC8H = _mk_layout()

OFF32 = {}
_c32 = 0
def _a32(name, cols):
    global _c32
    OFF32[name] = _c32
    _c32 += cols
_a32("b1", 5); _a32("b3", 5); _a32("bm2", 1); _a32("bh", 1)
_a32("rot", 9); _a32("qx", BC // 16); _a32("qxn", BC // 16)
NC32 = _c32


def _build_program():
    nc = bacc.Bacc("TRN2", target_bir_lowering=False, debug=False)

    xT_d = nc.dram_tensor("xT", [128, BC], FP8, kind="ExternalInput")
    wb8_d = nc.dram_tensor("wb8", [128, 2 * NC8H], FP8, kind="ExternalInput")
    wbb_d = nc.dram_tensor("wbb", [128, 96], BF16, kind="ExternalInput")
    wb32_d = nc.dram_tensor("wb32", [128, NC32], F32, kind="ExternalInput")
    ones_d = nc.dram_tensor("ones8", [1, CB], FP8, kind="ExternalInput")
    out_d = nc.dram_tensor("out", [3, BC], F32, kind="ExternalOutput")

    eng = {"a": None, "d": None, "p": None}  # filled after nc exists

    from contextlib import ExitStack
    with tile.TileContext(nc) as tc, ExitStack() as ctx:
        wts = ctx.enter_context(tc.tile_pool(name="wts", bufs=1))
        per = ctx.enter_context(tc.tile_pool(name="per", bufs=1))
        qp = ctx.enter_context(tc.tile_pool(name="qp", bufs=1))
        mxp = ctx.enter_context(tc.tile_pool(name="mxp", bufs=2))
        hdp = ctx.enter_context(tc.tile_pool(name="hdp", bufs=2))
        psA = ctx.enter_context(tc.tile_pool(name="psA", bufs=2, space="PSUM"))
        psB = ctx.enter_context(tc.tile_pool(name="psB", bufs=2, space="PSUM"))

        eng["a"], eng["d"], eng["p"] = nc.scalar, nc.vector, nc.gpsimd

        def extract(e, dst, src, bias=None):
            """dst = relu(src + bias), cast to dst dtype."""
            if e == "a":
                nc.scalar.activation(dst, src, AF.Relu,
                                     bias=(bias if bias is not None else 0.0))
            else:
                v = eng[e]
                if bias is not None:
                    v.tensor_scalar(dst, src, bias, 0.0, ALU.add, ALU.max)
                else:
                    v.tensor_scalar_max(dst, src, 0.0)

        # ---- weights ----
        # chunk 0 input first: conv1's critical path is xc + t1
        xcs = [per.tile([128, 2, CB], FP8, tag=f"xc{i}", name=f"xc{i}")
               for i in range(2)]
        nc.sync.dma_start(out=xcs[0][:, 0, :], in_=xT_d[:, 0:CB])
        wb8 = wts.tile([128, 2, NC8H], FP8, tag="wb8", name="wb8")
        t2_end = OFF8["t3_0_0"]
        # conv1+conv2 stationaries (both k-subtile halves) first
        nc.scalar.dma_start(out=wb8[:, 0, 0:t2_end], in_=wb8_d[:, 0:t2_end])
        nc.scalar.dma_start(out=wb8[:, 1, 0:t2_end],
                            in_=wb8_d[:, NC8H:NC8H + t2_end])
        nc.scalar.dma_start(out=wb8[:, 0, t2_end:NC8H],
                            in_=wb8_d[:, t2_end:NC8H])
        nc.scalar.dma_start(out=wb8[:, 1, t2_end:NC8H],
                            in_=wb8_d[:, NC8H + t2_end:2 * NC8H])
        wbb = wts.tile([128, 96], BF16, tag="wbb", name="wbb")
        nc.scalar.dma_start(out=wbb, in_=wbb_d[:, :])
        wb32 = wts.tile([128, NC32], F32, tag="wb32", name="wb32")
        nc.scalar.dma_start(out=wb32, in_=wb32_d[:, :])

        def v8(name, rows=128):
            o = OFF8[name]
            cols = {"t1": 576}.get(name)
            if cols is None:
                # single block width from layout
                nxt = sorted(v for v in OFF8.values() if v > o)
                cols = (nxt[0] if nxt else NC8H) - o
            return wb8[:rows, :, o:o + cols]

        t1v = wb8[:, :, 0:576]
        w2v = wbb[:, 0:64]
        w3v = wbb[:, 64:96]   # 3 real cols + 29 zero (fills PSUM partitions)
        b1v = lambda mj: wb32[:, OFF32["b1"] + mj:OFF32["b1"] + mj + 1]
        b3v = lambda mj: wb32[:, OFF32["b3"] + mj:OFF32["b3"] + mj + 1]
        bm2v = wb32[:64, OFF32["bm2"]:OFF32["bm2"] + 1]
        bhv = wb32[:, OFF32["bh"]:OFF32["bh"] + 1]
        rot = wb32[:, OFF32["rot"]:OFF32["rot"] + 9]
        qx_v = wb32[:, OFF32["qx"]:OFF32["qx"] + BC // 16]
        qxn_v = wb32[:, OFF32["qxn"]:OFF32["qxn"] + BC // 16]

        zc = wts.tile([128, 1], F32, tag="zc", name="zc")
        nc.vector.memset(zc, 0.0)
        halfpi = wts.tile([128, 1], F32, tag="halfpi", name="halfpi")
        nc.vector.memset(halfpi, float(np.pi / 2))

        # ---- persistent double-buffered activation tiles (padded rows) ----
        h1bigs = [per.tile([128, 5, CB], FP8, tag=f"h1b{i}", name=f"h1b{i}")
                  for i in range(2)]
        poolbigs = [per.tile([128, 3, CB], FP8, tag=f"plb{i}", name=f"plb{i}")
                    for i in range(2)]
        h3bigs = [per.tile([128, 5, CB], FP8, tag=f"h3b{i}", name=f"h3b{i}")
                  for i in range(2)]
        for i in range(2):
            # xc subtile-1 is all zero (conv1 contracts only subtile 0)
            nc.gpsimd.memset(xcs[i][:, 1, :], 0.0)
            # h1 k-tile 4: rows 64:128 pad; row 64 = 1.0 (conv2 bias row)
            nc.vector.memset(h1bigs[i][64:128, 4, :], 0.0)
            nc.vector.memset(h1bigs[i][64:65, 4, :], 1.0)
            # pool bands: rows 96:128 pad; band1 row 96 = 1.0 (conv3 bias row)
            nc.gpsimd.memset(poolbigs[i][96:128, 0, :], 0.0)
            nc.gpsimd.memset(poolbigs[i][96:128, 1, :], 0.0)
            nc.gpsimd.memset(poolbigs[i][96:128, 2, :], 0.0)
            nc.gpsimd.memset(poolbigs[i][96:97, 1, :], 1.0)
            # h3 k-tile 4: rows 72:128 pad; row 72 = 1.0 (mlp1 bias row).
            # rows 64:72 are re-written by the quantum DMAs each chunk;
            # the const-1 row lands at base 72 via DMA (compute ops need
            # 32-aligned partition bases).
            nc.vector.memset(h3bigs[i][64:128, 4, :], 0.0)
            nc.sync.dma_start(out=h3bigs[i][72:73, 4, :], in_=ones_d[:, :])

        # ---- quantum sim, qubit-interleaved [q + 8g, j], b = g*512 + j ----
        def emit_quantum():
            qst = None
            for l in range(3):
                sa = qp.tile([128, BC // 16], F32, tag="sa", name=f"sa{l}")
                ca = qp.tile([128, BC // 16], F32, tag="ca", name=f"ca{l}")
                nc.gpsimd.tensor_scalar_mul(sa, qx_v, rot[:, 3 * l:3 * l + 1])
                nc.gpsimd.tensor_scalar_mul(ca, qxn_v, rot[:, 3 * l + 1:3 * l + 2])
                nc.scalar.activation(sa, sa, AF.Sin, bias=zc)
                nc.scalar.activation(ca, ca, AF.Sin, bias=halfpi)
                sc = qp.tile([128, BC // 16], F32, tag="sc", name=f"sc{l}")
                nc.gpsimd.tensor_mul(sc, sa, ca)
                if qst is None:
                    qst = sc
                else:
                    ta = qp.tile([128, BC // 16], F32, tag="ta", name=f"ta{l}")
                    nc.gpsimd.tensor_scalar_mul(ta, qst, rot[:, 3 * l + 2:3 * l + 3])
                    nc.scalar.activation(ta, ta, AF.Tanh, bias=zc)
                    qn = qp.tile([128, BC // 16], F32, tag="qn", name=f"qn{l}")
                    nc.gpsimd.tensor_add(qn, sc, ta)
                    qst = qn
            qfb = qp.tile([128, BC // 16], FP8, tag="qfb", name="qfb")
            nc.gpsimd.tensor_copy(qfb, qst)
            return qfb

        def quantum_rows(c, qfb):
            h3big = h3bigs[c % 2]
            for g in range(4):
                nc.sync.dma_start(out=h3big[64:72, 4, g * 512:(g + 1) * 512],
                                  in_=qfb[(4 * c + g) * 8:(4 * c + g + 1) * 8, :])

        # ---- pipeline stages ----
        def stage_a(c, qfb):
            c0 = c * CB
            xc = xcs[c % 2]
            h1big = h1bigs[c % 2]
            poolbig = poolbigs[c % 2]
            if c > 0:
                nc.sync.dma_start(out=xc[:, 0, :], in_=xT_d[:, c0:c0 + CB])

            # conv1: one DR pair per m-tile (subtile 1 zero)
            for mj, (m0, m1) in enumerate(M1P):
                r = m1 - m0
                for h in range(2):
                    hb = slice(h * 1024, (h + 1) * 1024)
                    ps = psA.tile([128, 1024], F32, tag="psA", name="ps1")
                    for s in range(2):
                        g = 2 * h + s
                        sl = slice(g * NSL, (g + 1) * NSL)
                        pl = slice(s * NSL, (s + 1) * NSL)
                        nc.tensor.matmul(ps[:r, pl], t1v[:, :, m0:m1],
                                         xc[:, :, sl], start=True, stop=True,
                                         perf_mode=mybir.MatmulPerfMode.DoubleRow)
                    extract(ENG_C1[mj], h1big[:r, mj, hb], ps[:r], b1v(mj)[:r])

            # conv2 + fused maxpool (bias pre-added in PSUM via const row).
            # relu is monotone so it can run before max; every intermediate
            # is clamped >= 0, making the final max a plain max.
            for t in range(3):
                for h in range(2):
                    hb = slice(h * 1024, (h + 1) * 1024)
                    merged = []
                    for pp in range(2):   # parity pairs (0,1) and (2,3)
                        pss = []
                        for p in (2 * pp, 2 * pp + 1):
                            mj = p * 3 + t
                            ps = psB.tile([96, 1024], F32, tag="psB", name="ps2")
                            pairs = PAIRS2[mj]
                            for i, (k0, _, _, _) in enumerate(pairs):
                                st8 = v8(f"t2_{mj}_{i}", 128)
                                for s in range(2):
                                    g = 2 * h + s
                                    sl = slice(g * NSL, (g + 1) * NSL)
                                    pl = slice(s * NSL, (s + 1) * NSL)
                                    nc.tensor.matmul(
                                        ps[:96, pl], st8[:, :, :],
                                        h1big[:, k0:k0 + 2, sl],
                                        start=(i == 0), stop=(i == len(pairs) - 1),
                                        perf_mode=mybir.MatmulPerfMode.DoubleRow)
                            pss.append(ps)
                        se = mxp.tile([96, 1024], BF16, tag=f"s{pp}", name="se")
                        nc.scalar.activation(se, pss[0], AF.Relu)
                        m = mxp.tile([96, 1024], BF16, tag=f"m{pp}", name="me")
                        nc.vector.scalar_tensor_tensor(m, pss[1], 0.0, se,
                                                       ALU.max, ALU.max)
                        merged.append(m)
                    nc.vector.tensor_max(poolbig[:96, t, hb],
                                         merged[0], merged[1])
            if qfb is not None:
                quantum_rows(c, qfb)

        def stage_b(c):
            c0 = c * CB
            poolbig = poolbigs[c % 2]
            h3big = h3bigs[c % 2]
            amlp = hdp.tile([128, CB], BF16, tag="amlp", name="amlp")
            fmlp = hdp.tile([128, CB], BF16, tag="fmlp", name="fmlp")
            ob = hdp.tile([128, 512], F32, tag="ob", name="ob")

            # conv3
            for mj, (m0, m1) in enumerate(M3P):
                r = m1 - m0
                pairs = PAIRS3[mj]
                for h in range(2):
                    hb = slice(h * 1024, (h + 1) * 1024)
                    ps = psA.tile([128, 1024], F32, tag="psA", name="ps3")
                    for i, (k0, _, _, _) in enumerate(pairs):
                        st8 = v8(f"t3_{mj}_{i}", 128)
                        for s in range(2):
                            g = 2 * h + s
                            sl = slice(g * NSL, (g + 1) * NSL)
                            pl = slice(s * NSL, (s + 1) * NSL)
                            nc.tensor.matmul(ps[:r, pl], st8[:, :, :],
                                             poolbig[:, k0:k0 + 2, sl],
                                             start=(i == 0), stop=(i == len(pairs) - 1),
                                             perf_mode=mybir.MatmulPerfMode.DoubleRow)
                    extract(ENG_C3[mj], h3big[:r, mj, hb], ps[:r], b3v(mj)[:r])

            # mlp1 (bias via h3big const row)
            for mt, (m0, m1) in enumerate(MMP):
                r = m1 - m0
                for h in range(2):
                    hb = slice(h * 1024, (h + 1) * 1024)
                    ps = psA.tile([128, 1024], F32, tag="psA", name="psm")
                    for i, (k0, _, _, _) in enumerate(PAIRS_M):
                        st8 = v8(f"w1_{mt}_{i}", 128)
                        for s in range(2):
                            g = 2 * h + s
                            sl = slice(g * NSL, (g + 1) * NSL)
                            pl = slice(s * NSL, (s + 1) * NSL)
                            nc.tensor.matmul(ps[:r, pl], st8[:, :, :],
                                             h3big[:, k0:k0 + 2, sl],
                                             start=(i == 0), stop=(i == 2),
                                             perf_mode=mybir.MatmulPerfMode.DoubleRow)
                    if mt == 0:
                        extract(ENG_M1[0], amlp[:, hb], ps[:128])
                    else:
                        extract(ENG_M1[1], fmlp[64:128, hb], ps[:64])

            # w2 head hidden (bf16 single)
            for h in range(2):
                hb = slice(h * 1024, (h + 1) * 1024)
                ps = psA.tile([128, 1024], F32, tag="psA", name="psw2")
                for s in range(2):
                    g = 2 * h + s
                    sl = slice(g * NSL, (g + 1) * NSL)
                    pl = slice(s * NSL, (s + 1) * NSL)
                    nc.tensor.matmul(ps[:64, pl], w2v, amlp[:, sl],
                                     start=True, stop=True)
                extract(ENG_W2, fmlp[0:64, hb], ps[:64], bm2v)

            # w3: pack 4 col-slices at partition offsets 0/32/64/96, one tanh
            ps = psA.tile([128, 1024], F32, tag="psA", name="psw3")
            for s in range(4):
                sl = slice(s * NSL, (s + 1) * NSL)
                nc.tensor.matmul(ps[32 * s:32 * s + 32, 0:512], w3v, fmlp[:, sl],
                                 start=True, stop=True,
                                 tile_position=(0, 32 * s))
            nc.scalar.activation(ob, ps[:128, 0:512], AF.Tanh, bias=bhv, scale=0.5)
            nc.gpsimd.tensor_scalar(ob, ob, 0.5, 0.5, ALU.mult, ALU.add)
            for s in range(4):
                nc.sync.dma_start(out=out_d[:, c0 + s * 512:c0 + (s + 1) * 512],
                                  in_=ob[32 * s:32 * s + 3, :])

        stage_a(0, None)
        qfb = emit_quantum()
        quantum_rows(0, qfb)
        stage_a(1, qfb)
        stage_b(0)
        stage_a(2, qfb)
        stage_b(1)
        stage_a(3, qfb)
        stage_b(2)
        stage_b(3)

    nc.compile()
    return nc


def _prep_host(inputs):
    conv1_w = np.asarray(inputs["conv1_w"], np.float32)
    conv2_w = np.asarray(inputs["conv2_w"], np.float32)
    conv3_w = np.asarray(inputs["conv3_w"], np.float32)
    T1, T2, T3 = _build_conv_maps(conv1_w, conv2_w, conv3_w)

    # padded k-layouts matching on-device tiles
    T2p = np.zeros((640, 1152), np.float32)
    T2p[:576] = T2
    T3p = np.zeros((384, 576), np.float32)
    for b_ in range(3):
        T3p[b_ * 128:b_ * 128 + 96] = T3[b_ * 96:(b_ + 1) * 96]

    pt_w1 = np.asarray(inputs["pt_w1"], np.float32)
    cf_w1 = np.asarray(inputs["cf_w1"], np.float32)
    perm = np.empty(584, np.int64)
    for pos in range(9):
        for co in range(64):
            perm[pos * 64 + co] = co * 9 + pos
    perm[576:] = np.arange(576, 584)
    W1 = np.concatenate([pt_w1[perm], cf_w1[perm]], axis=1)  # [584, 192]
    W1p = np.zeros((640, 192), np.float32)
    W1p[:512] = W1[:512]
    W1p[512:584] = W1[512:]          # rows 512..575 conv tail, 576..583 quantum

    b2f = np.tile(np.asarray(inputs["conv2_b"], np.float32), 36)  # conv2 order
    bm1 = np.concatenate([np.asarray(inputs["pt_b1"], np.float32),
                          np.asarray(inputs["cf_b1"], np.float32)])  # [192]

    # ---- fp8 stationary block ----
    wb8 = np.zeros((128, 2, NC8H), np.float32)
    # t1: subtile0 = T1 padded to 128 rows; subtile1 = 0
    wb8[:108, 0, 0:576] = T1
    for mj in range(12):
        m0, m1 = M2P[mj]
        for i, (k0, ma, mb, bias) in enumerate(PAIRS2[mj]):
            o = OFF8[f"t2_{mj}_{i}"]
            for j, mask in ((0, ma), (1, mb)):
                if mask:
                    wb8[:, j, o:o + 96] = T2p[(k0 + j) * 128:(k0 + j + 1) * 128, m0:m1]
                if bias and (k0 + j) == 4:
                    wb8[64, j, o:o + 96] = b2f[m0:m1]
    for mj in range(5):
        m0, m1 = M3P[mj]
        b3f = np.tile(np.asarray(inputs["conv3_b"], np.float32), 9)
        for i, (k0, ma, mb, bias) in enumerate(PAIRS3[mj]):
            o = OFF8[f"t3_{mj}_{i}"]
            for j, mask in ((0, ma), (1, mb)):
                if mask:
                    wb8[:, j, o:o + (m1 - m0)] = \
                        T3p[(k0 + j) * 128:(k0 + j + 1) * 128, m0:m1]
                if bias and (k0 + j) == 1:
                    wb8[96, j, o:o + (m1 - m0)] = b3f[m0:m1]
    for mt in range(2):
        m0, m1 = MMP[mt]
        for i, (k0, ma, mb, bias) in enumerate(PAIRS_M):
            o = OFF8[f"w1_{mt}_{i}"]
            for j, mask in ((0, ma), (1, mb)):
                if mask:
                    wb8[:, j, o:o + (m1 - m0)] = \
                        W1p[(k0 + j) * 128:(k0 + j + 1) * 128, m0:m1]
                if bias and (k0 + j) == 4:
                    wb8[72, j, o:o + (m1 - m0)] = bm1[m0:m1]

    # ---- bf16 block: w2 | w3s ----
    wbb = np.zeros((128, 96), np.float32)
    wbb[:, 0:64] = np.asarray(inputs["pt_w2"], np.float32)
    W3 = np.zeros((128, 3), np.float32)
    W3[0:64, 0:2] = np.asarray(inputs["pt_w3"], np.float32)
    W3[64:128, 2] = np.asarray(inputs["cf_w2"], np.float32)[:, 0]
    S = np.zeros((3, 3), np.float32)
    S[:, 0] = (1, -1, 0); S[:, 1] = (-1, 1, 0); S[:, 2] = (0, 0, 1)
    wbb[:, 64:67] = W3 @ S

    # ---- f32 block ----
    def pack_bias2(bvec, total, ntile):
        full = np.zeros(ntile * 128, np.float32)
        full[:total] = bvec
        return full.reshape(ntile, 128).T.copy()

    wb32 = np.zeros((128, NC32), np.float32)
    b1 = pack_bias2(np.tile(np.asarray(inputs["conv1_b"], np.float32), 36), 576, 5)
    b3 = pack_bias2(np.tile(np.asarray(inputs["conv3_b"], np.float32), 9), 576, 5)
    wb32[:, OFF32["b1"]:OFF32["b1"] + 5] = b1
    wb32[:, OFF32["b3"]:OFF32["b3"] + 5] = b3
    wb32[:64, OFF32["bm2"]] = np.asarray(inputs["pt_b2"], np.float32)
    bh = np.concatenate([np.asarray(inputs["pt_b3"], np.float32),
                         np.asarray(inputs["cf_b2"], np.float32)]).reshape(3, 1)
    bhS = 0.5 * (S.T @ bh)[:, 0]
    for s in range(4):
        wb32[32 * s:32 * s + 3, OFF32["bh"]] = bhS

    qpar = np.asarray(inputs["quantum_params"], np.float32)  # [3,8,3]
    rot = np.zeros((128, 9), np.float32)
    for g in range(16):
        for q in range(8):
            for l in range(3):
                for i in range(3):
                    rot[q + 8 * g, l * 3 + i] = qpar[l, q, i]
    wb32[:, OFF32["rot"]:OFF32["rot"] + 9] = rot

    shared = {"wb8": np.ascontiguousarray(
                  wb8.reshape(128, 2 * NC8H)).astype(nf8),
              "wbb": wbb.astype(nbf),
              "ones8": np.ones((1, CB), np.float32).astype(nf8)}

    board = np.asarray(inputs["board_state"], np.float32).reshape(B, 108)
    in_maps = []
    for c in range(NCORES):
        bx = board[c * BC:(c + 1) * BC]          # [8192, 108]
        xq = bx[:, :NQ]
        xqn = np.roll(xq, -1, axis=1)
        m = dict(shared)
        xp = np.zeros((128, BC), np.float32)
        xp[:108] = bx.T
        m["xT"] = xp.astype(nf8)
        wb32c = wb32.copy()
        wb32c[:, OFF32["qx"]:OFF32["qx"] + BC // 16] = \
            xq.reshape(16, BC // 16, 8).transpose(0, 2, 1).reshape(128, BC // 16)
        wb32c[:, OFF32["qxn"]:OFF32["qxn"] + BC // 16] = \
            xqn.reshape(16, BC // 16, 8).transpose(0, 2, 1).reshape(128, BC // 16)
        m["wb32"] = wb32c
        in_maps.append(m)
    return in_maps


def kernel(**inputs):
    in_maps = _prep_host(inputs)
    if "nc" not in _cache:
        _cache["nc"] = _build_program()
    import os
    trace = os.environ.get("BASS_TRACE", "0") == "1"
    res = run_bass_kernel_spmd(_cache["nc"], in_maps, core_ids=list(range(NCORES)),
                               trace=trace)
    if res.exec_time_ns is not None:
        print(f"HW exec time: {res.exec_time_ns} ns")
        if res.instructions_and_trace is not None:
            print("trace:", res.instructions_and_trace[1])
    out = np.empty((B, 3), np.float32)
    for c in range(NCORES):
        out[c * BC:(c + 1) * BC] = res.results[c]["out"].T
    return out


if __name__ == "__main__":
    rng = np.random.default_rng(0)
    fake = {
        "board_state": rng.standard_normal((B, 3, 6, 6), dtype=np.float32),
        "target_positions": np.zeros((4, 2), np.int64),
        "conv1_w": rng.standard_normal((16, 3, 3, 3), dtype=np.float32) * 0.1,
        "conv1_b": rng.standard_normal(16, dtype=np.float32) * 0.1,
        "conv2_w": rng.standard_normal((32, 16, 3, 3), dtype=np.float32) * 0.05,
        "conv2_b": rng.standard_normal(32, dtype=np.float32) * 0.1,
        "conv3_w": rng.standard_normal((64, 32, 3, 3), dtype=np.float32) * 0.05,
        "conv3_b": rng.standard_normal(64, dtype=np.float32) * 0.1,
        "quantum_params": rng.standard_normal((3, 8, 3), dtype=np.float32),
        "pt_w1": rng.standard_normal((584, 128), dtype=np.float32) * 0.04,
        "pt_b1": rng.standard_normal(128, dtype=np.float32) * 0.04,
        "pt_w2": rng.standard_normal((128, 64), dtype=np.float32) * 0.09,
        "pt_b2": rng.standard_normal(64, dtype=np.float32) * 0.09,
        "pt_w3": rng.standard_normal((64, 2), dtype=np.float32) * 0.125,
        "pt_b3": rng.standard_normal(2, dtype=np.float32) * 0.125,
        "cf_w1": rng.standard_normal((584, 64), dtype=np.float32) * 0.04,
        "cf_b1": rng.standard_normal(64, dtype=np.float32) * 0.04,
        "cf_w2": rng.standard_normal((64, 1), dtype=np.float32) * 0.125,
        "cf_b2": rng.standard_normal(1, dtype=np.float32) * 0.125,
    }
    o = kernel(**fake)
    print(o.shape, o[:2])


# revision 21
# speedup vs baseline: 1.2953x; 1.2953x over previous
"""CQCNN piece estimator on 8 trn2 NeuronCores.

Strategy: pure data parallel over batch (8192 samples/core), SPMD (one NEFF).
Activations feature-major [features(partitions), batch(free)].
Convs are dense linear maps executed as fp8e4 DoubleRow matmuls (2 k-subtiles
per pass at 0.5 cycles/col) with zero-masked subtiles for odd k-tile counts.
conv2's bias rides a constant-1 row in the padded h1 tile so maxpool+bias+relu
collapses to 2 tensor_max + 1 scalar_tensor_tensor straight off PSUM.
Post-matmul work is spread across ACT / DVE / GPSIMD(Pool).
"""

import numpy as np
import ml_dtypes

import concourse.bass as bass
import concourse.bacc as bacc
import concourse.mybir as mybir
import concourse.tile as tile
from concourse.bass_utils import run_bass_kernel_spmd

BF16 = mybir.dt.bfloat16
F32 = mybir.dt.float32
FP8 = mybir.dt.float8e4
nbf = ml_dtypes.bfloat16
nf8 = ml_dtypes.float8_e4m3

B = 65536
NCORES = 8
BC = B // NCORES          # 8192 per core
CB = 2048                 # chunk of batch processed per pipeline pass
NCHUNK = BC // CB         # 4
NSL = 512                 # matmul moving-operand slice
NQ = 8

AF = mybir.ActivationFunctionType
ALU = mybir.AluOpType

_cache = {}

# ---- m-tile geometry ----
M1P = [(0, 128), (128, 256), (256, 384), (384, 512), (512, 576)]   # conv1 out
M2P = [(p * 288 + t * 96, p * 288 + (t + 1) * 96)
       for p in range(4) for t in range(3)]                        # conv2 out
M3P = [(0, 128), (128, 256), (256, 384), (384, 512), (512, 576)]   # conv3 out
MMP = [(0, 128), (128, 192)]                                       # mlp1 out

# engine assignment ('a'=ACT, 'd'=DVE, 'p'=Pool).
# Pool/GPSIMD cannot touch PSUM, and compute ops read at most ONE PSUM
# operand, so: ACT relu-copies even parities, DVE merges odd parities via
# scalar_tensor_tensor (1 PSUM input), Pool does the SBUF-only final max.
ENG_C1 = ['a', 'a', 'a', 'a', 'a']     # conv1 biasrelu per m-tile
ENG_C3 = ['a', 'd', 'a', 'd', 'a']     # conv3 biasrelu per m-tile
ENG_M1 = ['a', 'a']                    # mlp1 per m-tile
ENG_W2 = 'd'                           # w2 biasrelu


def _build_conv_maps(conv1_w, conv2_w, conv3_w):
    """Dense linear maps for the three convs (same orderings as baseline).

    X in-features  : channel-major c*36 + y*6 + x
    H1 out-features: pos-major (y*6+x)*16 + c
    H2 out-features: par*288 + q*32 + c   (par=(y%2)*2+x%2, q=(y//2)*3+x//2)
    P  (pooled)    : q*32 + c
    H3 out-features: pos-major (y*3+x)*64 + c
    """
    T1 = np.zeros((108, 576), np.float32)
    for co in range(16):
        for ci in range(3):
            for ky in range(3):
                for kx in range(3):
                    w = conv1_w[co, ci, ky, kx]
                    for yo in range(6):
                        yi = yo + ky - 1
                        if not 0 <= yi < 6:
                            continue
                        for xo in range(6):
                            xi = xo + kx - 1
                            if 0 <= xi < 6:
                                T1[ci * 36 + yi * 6 + xi, (yo * 6 + xo) * 16 + co] = w

    T2 = np.zeros((576, 1152), np.float32)
    for ky in range(3):
        for kx in range(3):
            w = conv2_w[:, :, ky, kx]  # [32,16]
            for yo in range(6):
                yi = yo + ky - 1
                if not 0 <= yi < 6:
                    continue
                for xo in range(6):
                    xi = xo + kx - 1
                    if 0 <= xi < 6:
                        par = (yo % 2) * 2 + (xo % 2)
                        qo = (yo // 2) * 3 + (xo // 2)
                        pi, po = (yi * 6 + xi) * 16, par * 288 + qo * 32
                        T2[pi:pi + 16, po:po + 32] = w.T

    T3 = np.zeros((288, 576), np.float32)
    for ky in range(3):
        for kx in range(3):
            w = conv3_w[:, :, ky, kx]  # [64,32]
            for yo in range(3):
                yi = yo + ky - 1
                if not 0 <= yi < 3:
                    continue
                for xo in range(3):
                    xi = xo + kx - 1
                    if 0 <= xi < 3:
                        pi, po = (yi * 3 + xi) * 32, (yo * 3 + xo) * 64
                        T3[pi:pi + 32, po:po + 64] = w.T
    return T1, T2, T3


def _pairs_from_run(ka, kb, kmax):
    """Pair a contiguous k-tile run [ka..kb] into DoubleRow (k0,k0+1) pairs.

    Returns [(k0, mask0, mask1)]; a masked subtile contributes zero weights
    (partner tile holds real data, so no NaN×0 risk)."""
    pairs = []
    i = ka
    while i <= kb:
        if i + 1 <= kb:
            pairs.append((i, 1, 1))
            i += 2
        else:
            if i + 1 <= kmax:
                pairs.append((i, 1, 0))
            else:
                pairs.append((i - 1, 0, 1))
            i += 1
    return pairs


def _conv2_pairs():
    """Per conv2 m-tile (p,t): DR pairs. Bias is applied during extraction
    (ACT bias slot / stt scalar slot), not in PSUM."""
    out = []
    for p in range(4):
        py = p // 2
        for t in range(3):
            yo = 2 * t + py
            a_pos, b_pos = 6 * max(0, yo - 1), 6 * min(5, yo + 1) + 5
            ka, kb = (a_pos * 16) // 128, (b_pos * 16 + 15) // 128
            pairs = [list(pr) + [False] for pr in _pairs_from_run(ka, kb, 4)]
            out.append(pairs)
    return out


def _conv3_pairs():
    # supports over 3 pool bands; bias rides row 96 of band 1 (pad const-1)
    out = []
    for mj in range(5):
        ys = {pos // 3 for pos in range(mj * 2, min(mj * 2 + 2, 9))}
        bands = sorted({b for y in ys for b in (y - 1, y, y + 1) if 0 <= b <= 2})
        pairs = [list(pr) + [False] for pr in _pairs_from_run(bands[0], bands[-1], 2)]
        carrier = next(pr for pr in pairs
                       if pr[0] == 1 or (pr[0] + 1 == 1))
        carrier[3] = True
        out.append(pairs)
    return out


PAIRS2 = _conv2_pairs()
PAIRS3 = _conv3_pairs()
PAIRS_M = [[0, 1, 1, False], [2, 1, 1, False], [3, 0, 1, True]]  # mlp1 k-pairs
# mlp1 bias rides row 72 of k-tile 4 (h3big pad const-1 row)


def _mk_layout():
    """fp8 stationary block offsets inside wb8 [128, 2, NC8H]."""
    off, c = {}, 0

    def a(name, cols):
        nonlocal c
        off[name] = c
        c += cols
    a("t1", 576)
    for mj in range(12):
        for i in range(len(PAIRS2[mj])):
            a(f"t2_{mj}_{i}", 96)
    for mj in range(5):
        for i in range(len(PAIRS3[mj])):
            a(f"t3_{mj}_{i}", M3P[mj][1] - M3P[mj][0])
    for mt in range(2):
        for i in range(3):
            a(f"w1_{mt}_{i}", MMP[mt][1] - MMP[mt][0])
    return off, c


OFF8, NC8H = _mk_layout()

OFF32 = {}
_c32 = 0
def _a32(name, cols):
    global _c32
    OFF32[name] = _c32
    _c32 += cols
_a32("b1", 5); _a32("b2", 1); _a32("b3", 5); _a32("bm2", 1); _a32("bh", 1)
_a32("rot", 9); _a32("qx", BC // 16); _a32("qxn", BC // 16)
NC32 = _c32


def _build_program():
    nc = bacc.Bacc("TRN2", target_bir_lowering=False, debug=False)

    xT_d = nc.dram_tensor("xT", [128, BC], FP8, kind="ExternalInput")
    wb8_d = nc.dram_tensor("wb8", [128, 2 * NC8H], FP8, kind="ExternalInput")
    wbb_d = nc.dram_tensor("wbb", [128, 96], BF16, kind="ExternalInput")
    wb32_d = nc.dram_tensor("wb32", [128, NC32], F32, kind="ExternalInput")
    ones_d = nc.dram_tensor("ones8", [1, CB], FP8, kind="ExternalInput")
    out_d = nc.dram_tensor("out", [3, BC], F32, kind="ExternalOutput")

    eng = {"a": None, "d": None, "p": None}  # filled after nc exists

    from contextlib import ExitStack
    with tile.TileContext(nc) as tc, ExitStack() as ctx:
        wts = ctx.enter_context(tc.tile_pool(name="wts", bufs=1))
        per = ctx.enter_context(tc.tile_pool(name="per", bufs=1))
        qp = ctx.enter_context(tc.tile_pool(name="qp", bufs=1))
        mxp = ctx.enter_context(tc.tile_pool(name="mxp", bufs=2))
        hdp = ctx.enter_context(tc.tile_pool(name="hdp", bufs=2))
        psA = ctx.enter_context(tc.tile_pool(name="psA", bufs=4, space="PSUM"))
        psB = psA

        eng["a"], eng["d"], eng["p"] = nc.scalar, nc.vector, nc.gpsimd

        def extract(e, dst, src, bias=None):
            """dst = relu(src + bias), cast to dst dtype."""
            if e == "a":
                nc.scalar.activation(dst, src, AF.Relu,
                                     bias=(bias if bias is not None else 0.0))
            else:
                v = eng[e]
                if bias is not None:
                    v.tensor_scalar(dst, src, bias, 0.0, ALU.add, ALU.max)
                else:
                    v.tensor_scalar_max(dst, src, 0.0)

        # ---- weights ----
        # chunk 0 input first: conv1's critical path is xc + t1
        xcs = [per.tile([128, 2, CB], FP8, tag=f"xc{i}", name=f"xc{i}")
               for i in range(2)]
        nc.sync.dma_start(out=xcs[0][:, 0, :], in_=xT_d[:, 0:CB])
        wb8 = wts.tile([128, 2, NC8H], FP8, tag="wb8", name="wb8")
        t2_end = OFF8["t3_0_0"]
        # conv1+conv2 stationaries (both k-subtile halves) first
        nc.scalar.dma_start(out=wb8[:, 0, 0:t2_end], in_=wb8_d[:, 0:t2_end])
        nc.scalar.dma_start(out=wb8[:, 1, 0:t2_end],
                            in_=wb8_d[:, NC8H:NC8H + t2_end])
        nc.scalar.dma_start(out=wb8[:, 0, t2_end:NC8H],
                            in_=wb8_d[:, t2_end:NC8H])
        nc.scalar.dma_start(out=wb8[:, 1, t2_end:NC8H],
                            in_=wb8_d[:, NC8H + t2_end:2 * NC8H])
        wbb = wts.tile([128, 96], BF16, tag="wbb", name="wbb")
        nc.scalar.dma_start(out=wbb, in_=wbb_d[:, :])
        wb32 = wts.tile([128, NC32], F32, tag="wb32", name="wb32")
        nc.scalar.dma_start(out=wb32, in_=wb32_d[:, :])

        def v8(name, rows=128):
            o = OFF8[name]
            cols = {"t1": 576}.get(name)
            if cols is None:
                # single block width from layout
                nxt = sorted(v for v in OFF8.values() if v > o)
                cols = (nxt[0] if nxt else NC8H) - o
            return wb8[:rows, :, o:o + cols]

        t1v = wb8[:, :, 0:576]
        w2v = wbb[:, 0:64]
        w3v = wbb[:, 64:96]   # 3 real cols + 29 zero (fills PSUM partitions)
        b1v = lambda mj: wb32[:, OFF32["b1"] + mj:OFF32["b1"] + mj + 1]
        b2v = wb32[:96, OFF32["b2"]:OFF32["b2"] + 1]
        b3v = lambda mj: wb32[:, OFF32["b3"] + mj:OFF32["b3"] + mj + 1]
        bm2v = wb32[:64, OFF32["bm2"]:OFF32["bm2"] + 1]
        bhv = wb32[:, OFF32["bh"]:OFF32["bh"] + 1]
        rot = wb32[:, OFF32["rot"]:OFF32["rot"] + 9]
        qx_v = wb32[:, OFF32["qx"]:OFF32["qx"] + BC // 16]
        qxn_v = wb32[:, OFF32["qxn"]:OFF32["qxn"] + BC // 16]

        zc = wts.tile([128, 1], F32, tag="zc", name="zc")
        nc.vector.memset(zc, 0.0)
        halfpi = wts.tile([128, 1], F32, tag="halfpi", name="halfpi")
        nc.vector.memset(halfpi, float(np.pi / 2))

        # ---- persistent double-buffered activation tiles (padded rows) ----
        h1bigs = [per.tile([128, 5, CB], FP8, tag=f"h1b{i}", name=f"h1b{i}")
                  for i in range(2)]
        poolbigs = [per.tile([128, 3, CB], FP8, tag=f"plb{i}", name=f"plb{i}")
                    for i in range(2)]
        h3bigs = [per.tile([128, 5, CB], FP8, tag=f"h3b{i}", name=f"h3b{i}")
                  for i in range(2)]
        for i in range(2):
            # xc subtile-1 is all zero (conv1 contracts only subtile 0)
            nc.gpsimd.memset(xcs[i][:, 1, :], 0.0)
            # h1 k-tile 4: rows 64:128 pad
            nc.vector.memset(h1bigs[i][64:128, 4, :], 0.0)
            # pool bands: rows 96:128 pad; band1 row 96 = 1.0 (conv3 bias row)
            nc.gpsimd.memset(poolbigs[i][96:128, 0, :], 0.0)
            nc.gpsimd.memset(poolbigs[i][96:128, 1, :], 0.0)
            nc.gpsimd.memset(poolbigs[i][96:128, 2, :], 0.0)
            nc.gpsimd.memset(poolbigs[i][96:97, 1, :], 1.0)
            # h3 k-tile 4: rows 72:128 pad; row 72 = 1.0 (mlp1 bias row).
            # rows 64:72 are re-written by the quantum DMAs each chunk;
            # the const-1 row lands at base 72 via DMA (compute ops need
            # 32-aligned partition bases).
            nc.vector.memset(h3bigs[i][64:128, 4, :], 0.0)
            nc.sync.dma_start(out=h3bigs[i][72:73, 4, :], in_=ones_d[:, :])

        # ---- quantum sim, qubit-interleaved [q + 8g, j], b = g*512 + j ----
        def emit_quantum():
            qst = None
            for l in range(3):
                sa = qp.tile([128, BC // 16], F32, tag="sa", name=f"sa{l}")
                ca = qp.tile([128, BC // 16], F32, tag="ca", name=f"ca{l}")
                nc.vector.tensor_scalar_mul(sa, qx_v, rot[:, 3 * l:3 * l + 1])
                nc.vector.tensor_scalar_mul(ca, qxn_v, rot[:, 3 * l + 1:3 * l + 2])
                nc.scalar.activation(sa, sa, AF.Sin, bias=zc)
                nc.scalar.activation(ca, ca, AF.Sin, bias=halfpi)
                sc = qp.tile([128, BC // 16], F32, tag="sc", name=f"sc{l}")
                nc.vector.tensor_mul(sc, sa, ca)
                if qst is None:
                    qst = sc
                else:
                    ta = qp.tile([128, BC // 16], F32, tag="ta", name=f"ta{l}")
                    nc.vector.tensor_scalar_mul(ta, qst, rot[:, 3 * l + 2:3 * l + 3])
                    nc.scalar.activation(ta, ta, AF.Tanh, bias=zc)
                    qn = qp.tile([128, BC // 16], F32, tag="qn", name=f"qn{l}")
                    nc.vector.tensor_add(qn, sc, ta)
                    qst = qn
            qfb = qp.tile([128, BC // 16], FP8, tag="qfb", name="qfb")
            nc.vector.tensor_copy(qfb, qst)
            return qfb

        def quantum_rows(c, qfb):
            h3big = h3bigs[c % 2]
            for g in range(4):
                nc.sync.dma_start(out=h3big[64:72, 4, g * 512:(g + 1) * 512],
                                  in_=qfb[(4 * c + g) * 8:(4 * c + g + 1) * 8, :])

        # ---- pipeline stages ----
        def stage_a(c, qfb):
            c0 = c * CB
            xc = xcs[c % 2]
            h1big = h1bigs[c % 2]
            poolbig = poolbigs[c % 2]
            if c > 0:
                nc.sync.dma_start(out=xc[:, 0, :], in_=xT_d[:, c0:c0 + CB])

            # conv1: one DR pair per m-tile (subtile 1 zero)
            for mj, (m0, m1) in enumerate(M1P):
                r = m1 - m0
                for h in range(2):
                    hb = slice(h * 1024, (h + 1) * 1024)
                    ps = psA.tile([128, 1024], F32, tag="psA", name="ps1")
                    for s in range(2):
                        g = 2 * h + s
                        sl = slice(g * NSL, (g + 1) * NSL)
                        pl = slice(s * NSL, (s + 1) * NSL)
                        nc.tensor.matmul(ps[:r, pl], t1v[:, :, m0:m1],
                                         xc[:, :, sl], start=True, stop=True,
                                         perf_mode=mybir.MatmulPerfMode.DoubleRow)
                    extract(ENG_C1[mj], h1big[:r, mj, hb], ps[:r], b1v(mj)[:r])

            # conv2 + fused maxpool: relu is monotone so it runs before max;
            # bias rides the ACT bias slot / stt scalar slot (same bias vector
            # for all four parities of a quadrant band).  Chain of depth 4:
            # s0=relu(p0+b); m_i = max(p_i + b, m_{i-1}) (all terms >= 0).
            for t in range(3):
                for h in range(2):
                    hb = slice(h * 1024, (h + 1) * 1024)
                    pss = []
                    for p in range(4):
                        mj = p * 3 + t
                        ps = psB.tile([128, 1024], F32, tag="psA", name="ps2")
                        pairs = PAIRS2[mj]
                        for i, (k0, _, _, _) in enumerate(pairs):
                            st8 = v8(f"t2_{mj}_{i}", 128)
                            for s in range(2):
                                g = 2 * h + s
                                sl = slice(g * NSL, (g + 1) * NSL)
                                pl = slice(s * NSL, (s + 1) * NSL)
                                nc.tensor.matmul(
                                    ps[:96, pl], st8[:, :, :],
                                    h1big[:, k0:k0 + 2, sl],
                                    start=(i == 0), stop=(i == len(pairs) - 1),
                                    perf_mode=mybir.MatmulPerfMode.DoubleRow)
                        pss.append(ps)
                    acc = mxp.tile([96, 1024], BF16, tag="acc0", name="acc0")
                    nc.scalar.activation(acc, pss[0][:96], AF.Relu, bias=b2v)
                    for p in range(1, 4):
                        last = (p == 3)
                        dst = (poolbig[:96, t, hb] if last else
                               mxp.tile([96, 1024], BF16, tag=f"acc{p}",
                                        name=f"acc{p}"))
                        nc.vector.scalar_tensor_tensor(dst, pss[p][:96], b2v,
                                                       acc, ALU.add, ALU.max)
                        acc = dst
            if qfb is not None:
                quantum_rows(c, qfb)

        def stage_b(c):
            c0 = c * CB
            poolbig = poolbigs[c % 2]
            h3big = h3bigs[c % 2]
            amlp = hdp.tile([128, CB], BF16, tag="amlp", name="amlp")
            fmlp = hdp.tile([128, CB], BF16, tag="fmlp", name="fmlp")
            ob = hdp.tile([128, 512], F32, tag="ob", name="ob")

            # conv3
            for mj, (m0, m1) in enumerate(M3P):
                r = m1 - m0
                pairs = PAIRS3[mj]
                for h in range(2):
                    hb = slice(h * 1024, (h + 1) * 1024)
                    ps = psA.tile([128, 1024], F32, tag="psA", name="ps3")
                    for i, (k0, _, _, _) in enumerate(pairs):
                        st8 = v8(f"t3_{mj}_{i}", 128)
                        for s in range(2):
                            g = 2 * h + s
                            sl = slice(g * NSL, (g + 1) * NSL)
                            pl = slice(s * NSL, (s + 1) * NSL)
                            nc.tensor.matmul(ps[:r, pl], st8[:, :, :],
                                             poolbig[:, k0:k0 + 2, sl],
                                             start=(i == 0), stop=(i == len(pairs) - 1),
                                             perf_mode=mybir.MatmulPerfMode.DoubleRow)
                    extract(ENG_C3[mj], h3big[:r, mj, hb], ps[:r], b3v(mj)[:r])

            # mlp1 (bias via h3big const row)
            for mt, (m0, m1) in enumerate(MMP):
                r = m1 - m0
                for h in range(2):
                    hb = slice(h * 1024, (h + 1) * 1024)
                    ps = psA.tile([128, 1024], F32, tag="psA", name="psm")
                    for i, (k0, _, _, _) in enumerate(PAIRS_M):
                        st8 = v8(f"w1_{mt}_{i}", 128)
                        for s in range(2):
                            g = 2 * h + s
                            sl = slice(g * NSL, (g + 1) * NSL)
                            pl = slice(s * NSL, (s + 1) * NSL)
                            nc.tensor.matmul(ps[:r, pl], st8[:, :, :],
                                             h3big[:, k0:k0 + 2, sl],
                                             start=(i == 0), stop=(i == 2),
                                             perf_mode=mybir.MatmulPerfMode.DoubleRow)
                    if mt == 0:
                        extract(ENG_M1[0], amlp[:, hb], ps[:128])
                    else:
                        extract(ENG_M1[1], fmlp[64:128, hb], ps[:64])

            # w2 head hidden (bf16 single)
            for h in range(2):
                hb = slice(h * 1024, (h + 1) * 1024)
                ps = psA.tile([128, 1024], F32, tag="psA", name="psw2")
                for s in range(2):
                    g = 2 * h + s
                    sl = slice(g * NSL, (g + 1) * NSL)
                    pl = slice(s * NSL, (s + 1) * NSL)
                    nc.tensor.matmul(ps[:64, pl], w2v, amlp[:, sl],
                                     start=True, stop=True)
                extract(ENG_W2, fmlp[0:64, hb], ps[:64], bm2v)

            # w3: pack 4 col-slices at partition offsets 0/32/64/96, one tanh
            ps = psA.tile([128, 1024], F32, tag="psA", name="psw3")
            for s in range(4):
                sl = slice(s * NSL, (s + 1) * NSL)
                nc.tensor.matmul(ps[32 * s:32 * s + 32, 0:512], w3v, fmlp[:, sl],
                                 start=True, stop=True,
                                 tile_position=(0, 32 * s))
            nc.scalar.activation(ob, ps[:128, 0:512], AF.Tanh, bias=bhv, scale=0.5)
            nc.vector.tensor_scalar(ob, ob, 0.5, 0.5, ALU.mult, ALU.add)
            for s in range(4):
                nc.sync.dma_start(out=out_d[:, c0 + s * 512:c0 + (s + 1) * 512],
                                  in_=ob[32 * s:32 * s + 3, :])

        stage_a(0, None)
        qfb = emit_quantum()
        quantum_rows(0, qfb)
        stage_a(1, qfb)
        stage_b(0)
        stage_a(2, qfb)
        stage_b(1)
        stage_a(3, qfb)
        stage_b(2)
        stage_b(3)

    nc.compile()
    return nc


def _prep_host(inputs):
    conv1_w = np.asarray(inputs["conv1_w"], np.float32)
    conv2_w = np.asarray(inputs["conv2_w"], np.float32)
    conv3_w = np.asarray(inputs["conv3_w"], np.float32)
    T1, T2, T3 = _build_conv_maps(conv1_w, conv2_w, conv3_w)

    # padded k-layouts matching on-device tiles
    T2p = np.zeros((640, 1152), np.float32)
    T2p[:576] = T2
    T3p = np.zeros((384, 576), np.float32)
    for b_ in range(3):
        T3p[b_ * 128:b_ * 128 + 96] = T3[b_ * 96:(b_ + 1) * 96]

    pt_w1 = np.asarray(inputs["pt_w1"], np.float32)
    cf_w1 = np.asarray(inputs["cf_w1"], np.float32)
    perm = np.empty(584, np.int64)
    for pos in range(9):
        for co in range(64):
            perm[pos * 64 + co] = co * 9 + pos
    perm[576:] = np.arange(576, 584)
    W1 = np.concatenate([pt_w1[perm], cf_w1[perm]], axis=1)  # [584, 192]
    W1p = np.zeros((640, 192), np.float32)
    W1p[:512] = W1[:512]
    W1p[512:584] = W1[512:]          # rows 512..575 conv tail, 576..583 quantum

    b2f = np.tile(np.asarray(inputs["conv2_b"], np.float32), 36)  # conv2 order
    bm1 = np.concatenate([np.asarray(inputs["pt_b1"], np.float32),
                          np.asarray(inputs["cf_b1"], np.float32)])  # [192]

    # ---- fp8 stationary block ----
    wb8 = np.zeros((128, 2, NC8H), np.float32)
    # t1: subtile0 = T1 padded to 128 rows; subtile1 = 0
    wb8[:108, 0, 0:576] = T1
    for mj in range(12):
        m0, m1 = M2P[mj]
        for i, (k0, ma, mb, bias) in enumerate(PAIRS2[mj]):
            o = OFF8[f"t2_{mj}_{i}"]
            for j, mask in ((0, ma), (1, mb)):
                if mask:
                    wb8[:, j, o:o + 96] = T2p[(k0 + j) * 128:(k0 + j + 1) * 128, m0:m1]
    for mj in range(5):
        m0, m1 = M3P[mj]
        b3f = np.tile(np.asarray(inputs["conv3_b"], np.float32), 9)
        for i, (k0, ma, mb, bias) in enumerate(PAIRS3[mj]):
            o = OFF8[f"t3_{mj}_{i}"]
            for j, mask in ((0, ma), (1, mb)):
                if mask:
                    wb8[:, j, o:o + (m1 - m0)] = \
                        T3p[(k0 + j) * 128:(k0 + j + 1) * 128, m0:m1]
                if bias and (k0 + j) == 1:
                    wb8[96, j, o:o + (m1 - m0)] = b3f[m0:m1]
    for mt in range(2):
        m0, m1 = MMP[mt]
        for i, (k0, ma, mb, bias) in enumerate(PAIRS_M):
            o = OFF8[f"w1_{mt}_{i}"]
            for j, mask in ((0, ma), (1, mb)):
                if mask:
                    wb8[:, j, o:o + (m1 - m0)] = \
                        W1p[(k0 + j) * 128:(k0 + j + 1) * 128, m0:m1]
                if bias and (k0 + j) == 4:
                    wb8[72, j, o:o + (m1 - m0)] = bm1[m0:m1]

    # ---- bf16 block: w2 | w3s ----
    wbb = np.zeros((128, 96), np.float32)
    wbb[:, 0:64] = np.asarray(inputs["pt_w2"], np.float32)
    W3 = np.zeros((128, 3), np.float32)
    W3[0:64, 0:2] = np.asarray(inputs["pt_w3"], np.float32)
    W3[64:128, 2] = np.asarray(inputs["cf_w2"], np.float32)[:, 0]
    S = np.zeros((3, 3), np.float32)
    S[:, 0] = (1, -1, 0); S[:, 1] = (-1, 1, 0); S[:, 2] = (0, 0, 1)
    wbb[:, 64:67] = W3 @ S

    # ---- f32 block ----
    def pack_bias2(bvec, total, ntile):
        full = np.zeros(ntile * 128, np.float32)
        full[:total] = bvec
        return full.reshape(ntile, 128).T.copy()

    wb32 = np.zeros((128, NC32), np.float32)
    b1 = pack_bias2(np.tile(np.asarray(inputs["conv1_b"], np.float32), 36), 576, 5)
    b3 = pack_bias2(np.tile(np.asarray(inputs["conv3_b"], np.float32), 9), 576, 5)
    wb32[:, OFF32["b1"]:OFF32["b1"] + 5] = b1
    wb32[:96, OFF32["b2"]] = b2f[0:96]
    wb32[:, OFF32["b3"]:OFF32["b3"] + 5] = b3
    wb32[:64, OFF32["bm2"]] = np.asarray(inputs["pt_b2"], np.float32)
    bh = np.concatenate([np.asarray(inputs["pt_b3"], np.float32),
                         np.asarray(inputs["cf_b2"], np.float32)]).reshape(3, 1)
    bhS = 0.5 * (S.T @ bh)[:, 0]
    for s in range(4):
        wb32[32 * s:32 * s + 3, OFF32["bh"]] = bhS

    qpar = np.asarray(inputs["quantum_params"], np.float32)  # [3,8,3]
    rot = np.zeros((128, 9), np.float32)
    for g in range(16):
        for q in range(8):
            for l in range(3):
                for i in range(3):
                    rot[q + 8 * g, l * 3 + i] = qpar[l, q, i]
    wb32[:, OFF32["rot"]:OFF32["rot"] + 9] = rot

    shared = {"wb8": np.ascontiguousarray(
                  wb8.reshape(128, 2 * NC8H)).astype(nf8),
              "wbb": wbb.astype(nbf),
              "ones8": np.ones((1, CB), np.float32).astype(nf8)}

    board = np.asarray(inputs["board_state"], np.float32).reshape(B, 108)
    in_maps = []
    for c in range(NCORES):
        bx = board[c * BC:(c + 1) * BC]          # [8192, 108]
        xq = bx[:, :NQ]
        xqn = np.roll(xq, -1, axis=1)
        m = dict(shared)
        xp = np.zeros((128, BC), np.float32)
        xp[:108] = bx.T
        m["xT"] = xp.astype(nf8)
        wb32c = wb32.copy()
        wb32c[:, OFF32["qx"]:OFF32["qx"] + BC // 16] = \
            xq.reshape(16, BC // 16, 8).transpose(0, 2, 1).reshape(128, BC // 16)
        wb32c[:, OFF32["qxn"]:OFF32["qxn"] + BC // 16] = \
            xqn.reshape(16, BC // 16, 8).transpose(0, 2, 1).reshape(128, BC // 16)
        m["wb32"] = wb32c
        in_maps.append(m)
    return in_maps


def kernel(**inputs):
    in_maps = _prep_host(inputs)
    if "nc" not in _cache:
        _cache["nc"] = _build_program()
    import os
    trace = os.environ.get("BASS_TRACE", "0") == "1"
    res = run_bass_kernel_spmd(_cache["nc"], in_maps, core_ids=list(range(NCORES)),
                               trace=trace)
    if res.exec_time_ns is not None:
        print(f"HW exec time: {res.exec_time_ns} ns")
        if res.instructions_and_trace is not None:
            print("trace:", res.instructions_and_trace[1])
    out = np.empty((B, 3), np.float32)
    for c in range(NCORES):
        out[c * BC:(c + 1) * BC] = res.results[c]["out"].T
    return out


if __name__ == "__main__":
    rng = np.random.default_rng(0)
    fake = {
        "board_state": rng.standard_normal((B, 3, 6, 6), dtype=np.float32),
        "target_positions": np.zeros((4, 2), np.int64),
        "conv1_w": rng.standard_normal((16, 3, 3, 3), dtype=np.float32) * 0.1,
        "conv1_b": rng.standard_normal(16, dtype=np.float32) * 0.1,
        "conv2_w": rng.standard_normal((32, 16, 3, 3), dtype=np.float32) * 0.05,
        "conv2_b": rng.standard_normal(32, dtype=np.float32) * 0.1,
        "conv3_w": rng.standard_normal((64, 32, 3, 3), dtype=np.float32) * 0.05,
        "conv3_b": rng.standard_normal(64, dtype=np.float32) * 0.1,
        "quantum_params": rng.standard_normal((3, 8, 3), dtype=np.float32),
        "pt_w1": rng.standard_normal((584, 128), dtype=np.float32) * 0.04,
        "pt_b1": rng.standard_normal(128, dtype=np.float32) * 0.04,
        "pt_w2": rng.standard_normal((128, 64), dtype=np.float32) * 0.09,
        "pt_b2": rng.standard_normal(64, dtype=np.float32) * 0.09,
        "pt_w3": rng.standard_normal((64, 2), dtype=np.float32) * 0.125,
        "pt_b3": rng.standard_normal(2, dtype=np.float32) * 0.125,
        "cf_w1": rng.standard_normal((584, 64), dtype=np.float32) * 0.04,
        "cf_b1": rng.standard_normal(64, dtype=np.float32) * 0.04,
        "cf_w2": rng.standard_normal((64, 1), dtype=np.float32) * 0.125,
        "cf_b2": rng.standard_normal(1, dtype=np.float32) * 0.125,
    }
    o = kernel(**fake)
    print(o.shape, o[:2])
